# revision 27
# baseline (speedup 1.0000x reference)
"""Multi-head attention (N=4, L=2048, C=1024, H=16, D=64) on 8 TRN2 NeuronCores.

Sharding: core c -> batch n = c//2, head-group g = c%2 (8 heads each).
Each core computes its 8 heads' attention + the partial output projection
for batch n; the host sums the two partials per batch and adds the
constant bias term (b_out + b_v @ W_out).

v2: all projections (qkv in, V, out) run as fp8e4 DoubleRow matmuls
(256-deep contraction per pass, half the PE streaming); weights are
pre-scaled x64 on the host to stay clear of fp8 subnormals, and the
4096x score scale / 4096x output scale are folded into the exp scale
and the final y copy. Inputs ship as fp8 (half the DMA), y returns bf16.
reciprocal -> reciprocal_approx_fast (~5x cheaper on DVE).

Device-side layout (per core):
  xT   [C=1024, L=2048]  fp8e4 (x[n].T, host-transposed/cast)
  wqk  [C, 1024]         fp8e4 (64*W_in cols: 8 heads' q dims then k dims)
  wv   [C, 512]          fp8e4 (64*W_in v cols)
  wo   [512, F=1024]     fp8e4 (64*W_out rows for the 8 heads)
  bqk  [128, 8]          f32   (64x q/k bias, partition-major per j-tile)
  y    [L, F]            bf16  output partial (scaled back by 1/4096)

Pipeline (ACT exp is the roofline engine; everything else hides under it):
  - qT/kT = W^T @ xT (j on partitions), V = xT^T @ Wv (l on partitions)
  - scoresT[k, q] per head, row-tiled head pairs (K=64 -> rows 0-63 /
    64-127, hardware-concurrent)
  - exp on ACT (scale=1/(8*4096) fused, fp32 PSUM -> bf16 SBUF)
  - AV^T col-tiled pairs into separate PSUM banks; row sums as M=64
    ones-matmuls (replicated across 64 partitions) cross-placed into the
    sibling head's free bank rows -> reciprocal and normalize are fully
    partition-aligned, no broadcast needed
  - qkT projections for the next pair and the final y projection are
    interleaved into the attention chunks as PE filler work
"""

import sys
from contextlib import ExitStack

import numpy as np

sys.path.insert(0, "/opt/trn_rl_repo")

import ml_dtypes

import concourse.bass as bass
import concourse.tile as tile
from concourse import bacc, mybir
from concourse.bass_utils import run_bass_kernel_spmd

BF16 = mybir.dt.bfloat16
F32 = mybir.dt.float32
FP8 = mybir.dt.float8e4
DRMODE = mybir.MatmulPerfMode.DoubleRow
FT = mybir.ActivationFunctionType
MULT = mybir.AluOpType.mult

N, L, C, H, D = 4, 2048, 1024, 16, 64
QKV = H * D  # 1024
F = 1024  # output feature dim
HG = 8  # heads per core
NCORES = 8
WS = 32.0  # host q/k weight prescale (fp8 subnormal avoidance)
SCALE = float(D) ** -0.5 / (WS * WS)  # exp scale: 0.125 / 4096

CT = C // 128  # 8 c-tiles
CP = CT // 2  # 4 DoubleRow c-tile pairs
LT = L // 128  # 16 l-tiles
JQ = L // 512  # 4 q-chunks
KT = L // 128  # 16 k-tiles
NP = HG // 2  # 4 head pairs

# Globals for test harness introspection
TRACE = False
DEBUG = False
LAST_RESULTS = None


def _build_program() -> bass.Bass:
    nc = bacc.Bacc()

    xT_d = nc.declare_dram_parameter("xT", [C, L], BF16, isOutput=False)
    x8_d = nc.declare_dram_parameter("x8", [C, L], FP8, isOutput=False)
    wqk_d = nc.declare_dram_parameter("wqk", [C, 1024], FP8, isOutput=False)
    wv_d = nc.declare_dram_parameter("wv", [C, 512], BF16, isOutput=False)
    wo_d = nc.declare_dram_parameter("wo", [512, F], BF16, isOutput=False)
    bqk_d = nc.declare_dram_parameter("bqk", [128, 8], F32, isOutput=False)
    y_d = nc.declare_dram_parameter("y", [L, F], BF16, isOutput=True)
    if DEBUG:
        dbg_qkT_d = nc.declare_dram_parameter("dbg_qkT", [128, 8, 4, 512], BF16, isOutput=True)
        dbg_V_d = nc.declare_dram_parameter("dbg_V", [128, LT, 512], BF16, isOutput=True)
        dbg_outT_d = nc.declare_dram_parameter("dbg_outT", [128, NP, L], FP8, isOutput=True)

    with tile.TileContext(nc) as tc, ExitStack() as ctx:
        const_pool = ctx.enter_context(tc.tile_pool(name="const", bufs=1))
        qk_pool = ctx.enter_context(tc.tile_pool(name="qkT", bufs=1))
        v_pool = ctx.enter_context(tc.tile_pool(name="V", bufs=1))
        outT_pool = ctx.enter_context(tc.tile_pool(name="outT", bufs=1))
        exp_pool = ctx.enter_context(tc.tile_pool(name="expT", bufs=2))
        r_pool = ctx.enter_context(tc.tile_pool(name="r", bufs=1))
        y_pool = ctx.enter_context(tc.tile_pool(name="y", bufs=2))
        wo_pool = ctx.enter_context(tc.tile_pool(name="wo", bufs=1))
        # PSUM: scores 2x2 banks + avA 1 + avB 1 + proj 2 = 8 banks
        ps_s = ctx.enter_context(tc.tile_pool(name="ps_s", bufs=2, space="PSUM"))
        ps_avA = ctx.enter_context(tc.tile_pool(name="ps_avA", bufs=1, space="PSUM"))
        ps_avB = ctx.enter_context(tc.tile_pool(name="ps_avB", bufs=1, space="PSUM"))
        ps_proj = ctx.enter_context(tc.tile_pool(name="ps_proj", bufs=1, space="PSUM"))

        ones64 = const_pool.tile([128, 64], BF16)
        nc.vector.memset(ones64[:], 1.0)
        bqk_sb = const_pool.tile([128, 8], F32)
        nc.sync.dma_start(bqk_sb[:], bqk_d[:])

        # qT/kT: [128, jt(8), jl(4), 512] ; jt 0-3 q dims, 4-7 k dims.
        # fp8: the score matmuls take fp8 operands in normal mode at full
        # speed; q/k noise only perturbs logits by ~0.006.
        qkT_sb = qk_pool.tile([128, 8, 4, 512], FP8)
        # V: [128, lt(16), 512]
        V_sb = v_pool.tile([128, LT, 512], BF16)
        # outT: [128, pair(4), L] (partitions = 2 heads x 64 dims)
        outT_sb = outT_pool.tile([128, NP, L], BF16)

        def qkT_proj_unit(xT_sb, wqk_sb, jt, lh):
            """qkT[j, l] = sum_c wqk[c, j] xT[c, l] for one (j-tile, L-half),
            as 4x2 DoubleRow matmuls (c-tile pairs)."""
            ps = ps_proj.tile([128, 2, 512], F32, tag="proj")
            for cp in range(CP):
                for lc in range(2):
                    nc.tensor.matmul(
                        ps[:, lc],
                        lhsT=wqk_sb[:, 2 * cp : 2 * cp + 2, jt * 128 : (jt + 1) * 128],
                        rhs=xT_sb[
                            :,
                            2 * cp : 2 * cp + 2,
                            lh * 1024 + lc * 512 : lh * 1024 + (lc + 1) * 512,
                        ],
                        start=(cp == 0),
                        stop=(cp == CP - 1),
                        perf_mode=DRMODE,
                    )
            nc.vector.tensor_scalar_add(
                qkT_sb[:, jt, 2 * lh : 2 * lh + 2, :], ps[:], bqk_sb[:, jt : jt + 1]
            )

        def score_kt(p, jq, expT, kt):
            """One k-tile of scoresT + its exp for head pair p, chunk jq."""
            S = ps_s.tile([128, 2, 512], F32, tag="s")
            jl, off = kt // 4, (kt % 4) * 128
            nc.tensor.matmul(
                S[:, 0],
                lhsT=qkT_sb[0:64, 4 + p, jl, off : off + 128],
                rhs=qkT_sb[0:64, p, jq, :],
                start=True,
                stop=True,
            )
            nc.tensor.matmul(
                S[:, 1],
                lhsT=qkT_sb[64:128, 4 + p, jl, off : off + 128],
                rhs=qkT_sb[64:128, p, jq, :],
                start=True,
                stop=True,
            )
            nc.scalar.activation(expT[:, kt], S[:], FT.Exp, scale=SCALE)

        def av_alloc():
            avA = ps_avA.tile([128, 512], F32, tag="avA")
            avB = ps_avB.tile([128, 512], F32, tag="avB")
            return avA, avB

        def av_mms(avA, avB, p, jq, expT, kts):
            """AV accumulation-group matmuls for the given k-tiles: both
            heads' AV into the avA bank (rows 0:64 / 64:128, col-tiled)."""
            hA, hB = 2 * p, 2 * p + 1
            for kt in kts:
                st, sp = kt == 0, kt == KT - 1
                nc.tensor.matmul(
                    avA[0:64],
                    lhsT=V_sb[:, kt, hA * 64 : hA * 64 + 64],
                    rhs=expT[:, kt, 0],
                    start=st,
                    stop=sp,
                )
                nc.tensor.matmul(
                    avA[64:128],
                    lhsT=V_sb[:, kt, hB * 64 : hB * 64 + 64],
                    rhs=expT[:, kt, 1],
                    start=st,
                    stop=sp,
                )

        def sum_mms(avA, avB, expT, kts):
            """Row sums, replicated across 64 partitions (M=64 ones), both
            heads into the avB bank (rows 0:64 / 64:128) so reciprocal and
            normalize run as single full-128-partition ops at base 0."""
            for kt in kts:
                st, sp = kt == 0, kt == KT - 1
                nc.tensor.matmul(
                    avB[0:64], lhsT=ones64[:], rhs=expT[:, kt, 0], start=st, stop=sp
                )
                nc.tensor.matmul(
                    avB[64:128], lhsT=ones64[:], rhs=expT[:, kt, 1], start=st, stop=sp
                )

        def norm_part(p, jq, avA, avB):
            # avA holds both heads' AV, avB both heads' sums. Copy AV to
            # SBUF (frees the bank), reciprocal straight off the sums bank,
            # then one fused normalize multiply into fp8 outT.
            stgAV = r_pool.tile([128, 512], F32, tag="stgA")
            nc.vector.tensor_copy(stgAV[:], avA[:])
            r_sb = r_pool.tile([128, 512], F32, tag="r")
            nc.vector.reciprocal_approx_fast(r_sb[:], avB[:])
            cols = slice(jq * 512, (jq + 1) * 512)
            nc.vector.tensor_tensor(
                outT_sb[:, p, cols], stgAV[:], r_sb[:], MULT
            )

        def y_unit_slices(lt):
            """y[l, f] = sum_d outT[d, l] wo[d, f] for one l-tile, split into
            two drippable half-slices (one per 512-wide f chunk); bf16 outT
            against fp8 wo (mixed dtypes are fine below fp32)."""
            box = {}

            def emit(fc, lt=lt):
                if fc == 0:
                    box["psy"] = ps_proj.tile(
                        [128, 2, 512], F32, tag="proj", name=f"psy_{lt}"
                    )
                    box["y"] = y_pool.tile([128, 1024], BF16, tag="y", name=f"y_{lt}")
                psy, y_sb = box["psy"], box["y"]
                for p in range(NP):
                    nc.tensor.matmul(
                        psy[:, fc],
                        lhsT=outT_sb[:, p, lt * 128 : (lt + 1) * 128],
                        rhs=wo_sb[:, p, fc * 512 : (fc + 1) * 512],
                        start=(p == 0),
                        stop=(p == NP - 1),
                    )
                nc.vector.tensor_copy(y_sb[:, fc * 512 : (fc + 1) * 512], psy[:, fc])
                if fc == 1:
                    # gpsimd queue: keeps output DMAs off the input-DMA queue
                    nc.gpsimd.dma_start(y_d[lt * 128 : (lt + 1) * 128, :], y_sb[:])

            return [lambda fc=fc: emit(fc) for fc in range(2)]

        with tc.tile_pool(name="xw", bufs=1) as xw_pool:
            # DMA order = critical-path order: wqk + x8 (gate the first q/k
            # projections and scores), then bf16 x + wv (gate only the V
            # projection, which runs later).
            xT_sb = xw_pool.tile([128, CT, L], BF16)
            x8_sb = xw_pool.tile([128, CT, L], FP8)
            wqk_sb = xw_pool.tile([128, CT, 1024], FP8)
            xT_r = xT_d.rearrange("(t p) l -> p t l", p=128)
            x8_r = x8_d.rearrange("(t p) l -> p t l", p=128)
            wqk_r = wqk_d.rearrange("(t p) j -> p t j", p=128)
            for ct in range(CT):
                nc.sync.dma_start(wqk_sb[:, ct], wqk_r[:, ct])
                nc.sync.dma_start(x8_sb[:, ct], x8_r[:, ct])

            def V_proj_unit(wv_sb, lt, pool_tag=None):
                if pool_tag is None:
                    pool_tag = (ps_avA, "avA") if lt % 2 == 0 else (ps_avB, "avB")
                pool, tag = pool_tag
                psv = pool.tile([128, 512], F32, tag=tag)
                for ct in range(CT):
                    nc.tensor.matmul(
                        psv[:],
                        lhsT=xT_sb[:, ct, lt * 128 : (lt + 1) * 128],
                        rhs=wv_sb[:, ct, :],
                        start=(ct == 0),
                        stop=(ct == CT - 1),
                    )
                nc.vector.tensor_copy(V_sb[:, lt, :], psv[:])

            def qkT_unit_slices(jt, lh):
                """A qkT projection unit split into 4 drippable slices of
                2 DoubleRow matmuls (the psum group spans the slices)."""
                box = {}

                def emit(i, jt=jt, lh=lh):
                    if i == 0:
                        box["ps"] = ps_proj.tile(
                            [128, 2, 512], F32, tag="proj", name=f"proj_{jt}_{lh}"
                        )
                    ps = box["ps"]
                    for lc in range(2):
                        nc.tensor.matmul(
                            ps[:, lc],
                            lhsT=wqk_sb[
                                :, 2 * i : 2 * i + 2, jt * 128 : (jt + 1) * 128
                            ],
                            rhs=x8_sb[
                                :,
                                2 * i : 2 * i + 2,
                                lh * 1024 + lc * 512 : lh * 1024 + (lc + 1) * 512,
                            ],
                            start=(i == 0),
                            stop=(i == CP - 1),
                            perf_mode=DRMODE,
                        )
                    if i == 3:
                        nc.vector.tensor_scalar_add(
                            qkT_sb[:, jt, 2 * lh : 2 * lh + 2, :],
                            ps[:],
                            bqk_sb[:, jt : jt + 1],
                        )

                return [lambda i=i: emit(i) for i in range(4)]

            # Emission schedule: per chunk c we emit its AV groups (paced by
            # its exps), then the first 4 score k-tiles of chunk c+1 woven
            # between the two halves of c's row-sum pass (the sums can only
            # start once the AV groups close, i.e. after c's last exp), then
            # the normalize, then the remaining score k-tiles of c+1 with
            # projection work dripped one slice per k-tile.
            with tc.tile_pool(name="wv", bufs=1) as wv_pool:
                wv_sb = wv_pool.tile([128, CT, 512], BF16)
                nc.sync.dma_start(wv_sb[:], wv_d.rearrange("(t p) j -> p t j", p=128))
                for ct in range(CT):
                    nc.sync.dma_start(xT_sb[:, ct], xT_r[:, ct])

                # pair 0 q/k projections up front — only the k-tiles 0-7 half
                # (units (4,0)+(0,0)) gates the first scores, so ACT starts
                # before unit (4,1) is even emitted.
                chunks = [(p, jq) for p in range(NP) for jq in range(JQ)]
                qkT_proj_unit(x8_sb, wqk_sb, 4, 0)
                qkT_proj_unit(x8_sb, wqk_sb, 0, 0)
                exp0 = exp_pool.tile([128, KT, 2, 512], BF16, tag="expT")
                for kt in range(8):
                    score_kt(0, 0, exp0, kt)
                qkT_proj_unit(x8_sb, wqk_sb, 4, 1)
                qk01 = qkT_unit_slices(0, 1)
                for kt in range(8, KT):
                    score_kt(0, 0, exp0, kt)
                    if kt % 2 == 0 and qk01:
                        qk01.pop(0)()
                while qk01:
                    qk01.pop(0)()
                # chunk (0,1) scores run clean (no V drip: the in-order PE
                # queue would stall scores behind the late bf16-x DMA)
                exp1 = exp_pool.tile([128, KT, 2, 512], BF16, tag="expT")
                for kt in range(KT):
                    score_kt(0, 1, exp1, kt)
                # first half of the V projection; the rest interleaves with
                # chunk (0,0)'s AV below
                for lt in range(8):
                    V_proj_unit(wv_sb, lt)

                exps = {0: exp0, 1: exp1}
                # chunk (0,0) inline: V units 8-15 woven into its AV phase
                p, jq = chunks[0]
                expT = exps.pop(0)
                avA, avB = av_alloc()
                for kt in range(KT):
                    if kt >= 8:
                        # ps_avA/avB hold this chunk's accumulators; route
                        # the V psum through the proj pool instead
                        V_proj_unit(wv_sb, kt, pool_tag=(ps_proj, "proj"))
                    av_mms(avA, avB, p, jq, expT, [kt])
                sum_mms(avA, avB, expT, range(0, KT))
                norm_part(p, jq, avA, avB)
                drip = qkT_unit_slices(5, 0)
                while drip:
                    drip.pop(0)()

            # wo loads after the prologue peak (first needed at chunk 12),
            # reusing the SBUF freed by the wv pool
            wo_sb = wo_pool.tile([128, 4, F], BF16)
            nc.sync.dma_start(wo_sb[:], wo_d.rearrange("(t p) f -> p t f", p=128))

            for ci in range(1, len(chunks)):
                p, jq = chunks[ci]
                nxt = chunks[ci + 1] if ci + 1 < len(chunks) else None
                emit_nxt = nxt is not None and (ci + 1) not in exps
                if emit_nxt:
                    exps[ci + 1] = exp_pool.tile([128, KT, 2, 512], BF16, tag="expT", name=f"expT_{ci+1}")
                expT = exps.pop(ci)
                avA, avB = av_alloc()
                av_mms(avA, avB, p, jq, expT, range(0, KT - 1))
                if emit_nxt:
                    # runs during this chunk's last exp (S slot frees at kt14)
                    score_kt(*nxt, exps[ci + 1], 0)
                av_mms(avA, avB, p, jq, expT, [KT - 1])
                if emit_nxt:
                    score_kt(*nxt, exps[ci + 1], 1)
                sum_mms(avA, avB, expT, range(0, 6))
                if emit_nxt:
                    score_kt(*nxt, exps[ci + 1], 2)
                sum_mms(avA, avB, expT, range(6, 11))
                if emit_nxt:
                    score_kt(*nxt, exps[ci + 1], 3)
                sum_mms(avA, avB, expT, range(11, KT))
                if emit_nxt:
                    score_kt(*nxt, exps[ci + 1], 4)
                norm_part(p, jq, avA, avB)

                # filler: next pair's projections (pairs 0-2) or the
                # output projection (pair 3), dripped per score k-tile
                if p < NP - 1:
                    nj = p + 1
                    jt, lh = [(4 + nj, 0), (4 + nj, 1), (nj, 0), (nj, 1)][jq]
                    drip = qkT_unit_slices(jt, lh)
                else:
                    drip = []
                    for lt in range(4 * jq, 4 * jq + 4):
                        drip.extend(y_unit_slices(lt))
                for kt in range(5, KT):
                    if emit_nxt:
                        score_kt(*nxt, exps[ci + 1], kt)
                    if drip:
                        drip.pop(0)()
                while drip:
                    drip.pop(0)()

            if DEBUG:
                nc.gpsimd.dma_start(dbg_qkT_d[:], qkT_sb[:])
                nc.gpsimd.dma_start(dbg_V_d[:], V_sb[:])
                nc.gpsimd.dma_start(dbg_outT_d[:], outT_sb[:])

    nc.finalize()
    return nc


_NC_CACHE = None


def _get_program():
    global _NC_CACHE
    if _NC_CACHE is None:
        _NC_CACHE = _build_program()
    return _NC_CACHE


def _make_in_maps(x, W_in, b_in, W_out):
    f8 = ml_dtypes.float8_e4m3
    bf = ml_dtypes.bfloat16
    in_maps = []
    for c in range(NCORES):
        n, g = c // 2, c % 2
        h0 = g * HG  # first global head
        j0 = h0 * D  # 512*g
        xT = np.ascontiguousarray(x[n].T).astype(bf)  # [C, L]
        x8 = xT.astype(f8)
        wqk = (
            WS
            * np.concatenate(
                [W_in[:, j0 : j0 + 512], W_in[:, QKV + j0 : QKV + j0 + 512]], axis=1
            )
        ).astype(f8)
        wv = np.ascontiguousarray(W_in[:, 2 * QKV + j0 : 2 * QKV + j0 + 512]).astype(bf)
        wo = np.ascontiguousarray(W_out[j0 : j0 + 512, :]).astype(bf)
        bqk = (
            (WS * np.concatenate([b_in[j0 : j0 + 512], b_in[QKV + j0 : QKV + j0 + 512]]))
            .astype(np.float32)
            .reshape(8, 128)
            .T.copy()
        )
        in_maps.append(
            {"xT": xT, "x8": x8, "wqk": wqk, "wv": wv, "wo": wo, "bqk": bqk}
        )
    return in_maps


def kernel(x, W_in, b_in, W_out, b_out):
    global LAST_RESULTS
    x = np.asarray(x, dtype=np.float32)
    W_in = np.asarray(W_in, dtype=np.float32)
    b_in = np.asarray(b_in, dtype=np.float32)
    W_out = np.asarray(W_out, dtype=np.float32)
    b_out = np.asarray(b_out, dtype=np.float32)

    nc = _get_program()
    in_maps = _make_in_maps(x, W_in, b_in, W_out)
    res = run_bass_kernel_spmd(nc, in_maps, list(range(NCORES)), trace=TRACE)
    LAST_RESULTS = res

    # host bias: b_out + b_v @ W_out  (b_v enters linearly through the
    # softmax-normalized value average: A@(V+b_v) = A@V + b_v)
    host_bias = (
        b_out.astype(np.float64)
        + b_in[2 * QKV :].astype(np.float64) @ W_out.astype(np.float64)
    ).astype(np.float32)

    out = np.empty((N, L, F), dtype=np.float32)
    for n in range(N):
        y0 = np.asarray(res.results[2 * n]["y"], dtype=np.float32)
        y1 = np.asarray(res.results[2 * n + 1]["y"], dtype=np.float32)
        out[n] = y0 + y1 + host_bias
    return out
